# revision 1
# baseline (speedup 1.0000x reference)
"""Trainium2 Bass kernel: 4-branch GNN message passing (scatter-mean propagation).

Strategy (8 NeuronCores, SPMD):
  - Nodes are sharded across the 8 cores (4096 dest nodes per core); the
    small weight matrices are replicated.
  - Branch i of the reference needs i scatter-mean propagations.  The six
    propagations are restructured into 3 stacked passes over column blocks:
        pass 1: [h1|h2|h3] (384 cols), pass 2: [h2'|h3'] (256), pass 3: [h3''].
  - Before each pass the per-core rows are AllGather'd so every core holds the
    full [32768, C] operand; each core then computes its own 4096 dest rows:
    a batched dma_gather pulls the source rows of its edges (sorted by dest,
    padded to uniform 128-edge tiles per 128-dest window), a one-hot matrix
    built on-device (iota + is_equal) turns the segment-sum into TF32
    (float32r) matmuls accumulated in PSUM, and the flush applies 1/deg.
  - The per-branch MLPs, gates, concat and output projection are local,
    computed feature-major so biases are per-partition scalars; gates are
    folded into Wo on the host.  The final output is returned feature-major
    per core and transposed/concatenated on the host.
"""

import os

import numpy as np

N = 32768
E = 524288
IN_F = 256
OUT_F = 128
NB = 4
NCORES = 8
LOCAL = N // NCORES          # 4096
P = 128
NWIN = LOCAL // P            # 32 dest windows per core
C1, C2, C3 = 3 * OUT_F, 2 * OUT_F, OUT_F
NODE_CHUNK = 512

LAST_EXEC_NS = None
_PROG_CACHE = {}


def _install_ntff_hook():
    """Make run_bass_kernel_spmd(trace=True) work when antenv.axon_hooks is absent."""
    import sys
    import types

    try:
        import antenv.axon_hooks  # noqa: F401
        return
    except ImportError:
        pass
    try:
        from trn_agent_boot.trn_boot import _ntff_profile_via_ctypes
    except ImportError:
        return
    hook = _ntff_profile_via_ctypes("/opt/axon/libaxon_pjrt.so")
    mod = types.ModuleType("antenv.axon_hooks")
    mod.get_axon_ntff_profile_hook = lambda: hook
    mod.set_axon_ntff_profile_hook = lambda h: None
    sys.modules["antenv.axon_hooks"] = mod


def _build_program(T, fpos_flat, call_kmax):
    import concourse.bacc as bacc
    import concourse.mybir as mybir
    import concourse.tile as tile
    from concourse.library_config import mlp as mlp_lib

    f32 = mybir.dt.float32
    f32r = mybir.dt.float32r
    bf16 = mybir.dt.bfloat16
    f8 = mybir.dt.float8e4
    i16 = mybir.dt.int16
    C1G = 512  # hg1 row width in f8 (384 used + 128 pad; 512B rows for dma_gather)

    nc = bacc.Bacc("TRN2", target_bir_lowering=False, num_swdge_queues=4)

    # ---- per-core external inputs -------------------------------------------------
    xt = nc.dram_tensor("xt", [2, P, LOCAL], bf16, kind="ExternalInput")
    gidx = nc.dram_tensor("gidx", [P, NWIN * T * 8], i16, kind="ExternalInput")
    destl = nc.dram_tensor("destl", [P, NWIN * T], bf16, kind="ExternalInput")
    iota_b = nc.dram_tensor("iota_b", [P, P], bf16, kind="ExternalInput")
    invd = nc.dram_tensor("invd", [P, NWIN], f32, kind="ExternalInput")
    iota_t = nc.dram_tensor("iota_t", [P, P], f32, kind="ExternalInput")
    iota_c = nc.dram_tensor("iota_c", [P, 1], f32, kind="ExternalInput")
    wb = nc.dram_tensor("wb", [2, P, NB * OUT_F], bf16, kind="ExternalInput")
    bbr = nc.dram_tensor("bbr", [1, NB * OUT_F], bf16, kind="ExternalInput")
    w1s = nc.dram_tensor("w1s", [P, NB * P], bf16, kind="ExternalInput")
    b1s = nc.dram_tensor("b1s", [P, NB], f32, kind="ExternalInput")
    w2s = nc.dram_tensor("w2s", [P, NB * P], bf16, kind="ExternalInput")
    b2s = nc.dram_tensor("b2s", [P, NB], f32, kind="ExternalInput")
    wos = nc.dram_tensor("wos", [P, NB * P], bf16, kind="ExternalInput")
    boc = nc.dram_tensor("boc", [P, 1], f32, kind="ExternalInput")

    outT = nc.dram_tensor("outT", [P, LOCAL], f32, kind="ExternalOutput")

    with tile.TileContext(nc) as tc:
        nc.gpsimd.load_library(mlp_lib)

        # ---- internal DRAM buffers -----------------------------------------------
        NAGC = 4
        QR = LOCAL // NAGC
        hb1q = [nc.dram_tensor(f"hb1q{k}", [QR, C1G], f8) for k in range(NAGC)]
        hg1 = nc.dram_tensor("hg1", [N, C1G], f8, addr_space="Shared")
        hb2q = [nc.dram_tensor(f"hb2q{k}", [QR, C2], f8) for k in range(NAGC)]
        hg2 = nc.dram_tensor("hg2", [N, C2], f8, addr_space="Shared")
        C3G = 256  # f8 rows padded to 256B for dma_gather
        hb3q = [nc.dram_tensor(f"hb3q{k}", [QR, C3G], f8) for k in range(NAGC)]
        hg3 = nc.dram_tensor("hg3", [N, C3G], f8, addr_space="Shared")


        with (
            tc.tile_pool(name="const", bufs=1) as cpool,
            tc.tile_pool(name="work", bufs=2) as work,
            tc.tile_pool(name="gath", bufs=18) as gpool,
            tc.tile_pool(name="psA", bufs=4, space="PSUM") as psA,
            tc.tile_pool(name="psB", bufs=1, space="PSUM") as psB,
        ):
            # ---- resident constants ----------------------------------------------
            gidx_sb = cpool.tile([P, NWIN * T * 8], i16)
            nc.sync.dma_start(gidx_sb[:], gidx[:])
            destl_sb = cpool.tile([P, NWIN * T], bf16)
            iotab_sb = cpool.tile([P, P], bf16)
            nc.sync.dma_start(iotab_sb[:], iota_b[:])
            nc.sync.dma_start(destl_sb[:], destl[:])
            invd_sb = cpool.tile([P, NWIN], f32)
            nc.sync.dma_start(invd_sb[:], invd[:])
            iota_sb = cpool.tile([P, P], f32)
            nc.sync.dma_start(iota_sb[:], iota_t[:])
            iotac_sb = cpool.tile([P, 1], f32)
            nc.sync.dma_start(iotac_sb[:], iota_c[:])
            wb_sb = cpool.tile([P, 2 * NB * OUT_F], bf16)
            nc.sync.dma_start(wb_sb[:, : NB * OUT_F], wb[0])
            nc.sync.dma_start(wb_sb[:, NB * OUT_F :], wb[1])
            bb_sb = cpool.tile([1, NB * OUT_F], bf16)
            nc.sync.dma_start(bb_sb[:], bbr[:])
            w1_sb = cpool.tile([P, NB * P], bf16)
            nc.sync.dma_start(w1_sb[:], w1s[:])
            w2_sb = cpool.tile([P, NB * P], bf16)
            nc.sync.dma_start(w2_sb[:], w2s[:])
            wo_sb = cpool.tile([P, NB * P], bf16)
            nc.sync.dma_start(wo_sb[:], wos[:])
            b1_sb = cpool.tile([P, NB], f32)
            nc.sync.dma_start(b1_sb[:], b1s[:])
            b2_sb = cpool.tile([P, NB], f32)
            nc.sync.dma_start(b2_sb[:], b2s[:])
            bo_sb = cpool.tile([P, 1], f32)
            nc.sync.dma_start(bo_sb[:], boc[:])

            # identity (for PE transpose) and an all-ones row (for bias matmuls)
            ident = cpool.tile([P, P], f32r)
            nc.vector.tensor_tensor(
                out=ident[:],
                in0=iotac_sb[:].to_broadcast([P, P]),
                in1=iota_sb[:],
                op=mybir.AluOpType.is_equal,
            )
            ones_sb = cpool.tile([1, P], bf16)
            nc.vector.tensor_tensor(
                out=ones_sb[:],
                in0=iota_sb[0:1, :],
                in1=iota_sb[0:1, :],
                op=mybir.AluOpType.is_equal,
            )

            # ---- phase 0: h = x @ Wb + bb for all 4 branches ---------------------
            AGC = 4                      # AllGather chunks per pass
            WPC = NWIN // AGC            # windows per AG chunk
            QROW = LOCAL // AGC          # rows per chunk (1024)
            rg = [list(range(NCORES))]

            def ag_chunk(srcq, dst, k, nk=1):
                R = NCORES * QROW
                nc.gpsimd.collective_compute(
                    "AllGather",
                    mybir.AluOpType.bypass,
                    replica_groups=rg,
                    ins=[srcq[k][:]],
                    outs=[dst[k * R : (k + 1) * R, :]],
                )

            def flush_mlp_cols(psw_or_ps0, col0, w, hTb):
                """inv_deg-scale cols [col0:col0+P], transpose, store feature-major."""
                sc = work.tile([P, P], f32r, tag="sc")
                nc.vector.tensor_scalar_mul(
                    sc[:], psw_or_ps0[:, col0 : col0 + P], invd_sb[:, w : w + 1]
                )
                pst = psB.tile([P, P], f32r, tag="pst")
                nc.tensor.transpose(out=pst[:], in_=sc[:], identity=ident[:])
                nc.vector.tensor_copy(hTb[:, w * P : (w + 1) * P], pst[:])

            def mlp_branch(i, hTb):
                for ch in range(LOCAL // NODE_CHUNK):
                    csl = slice(ch * NODE_CHUNK, (ch + 1) * NODE_CHUNK)
                    zp = psB.tile([P, NODE_CHUNK], f32, tag="zp", bufs=1)
                    nc.tensor.matmul(
                        zp[:], lhsT=w1_sb[:, i * P : (i + 1) * P], rhs=hTb[:, csl],
                        start=True, stop=True,
                    )
                    zr = work.tile([P, NODE_CHUNK], bf16, tag="zr")
                    nc.scalar.activation(
                        zr[:], zp[:], mybir.ActivationFunctionType.Relu,
                        bias=b1_sb[:, i : i + 1],
                    )
                    yp = psB.tile([P, NODE_CHUNK], f32, tag="yp")
                    nc.tensor.matmul(
                        yp[:], lhsT=w2_sb[:, i * P : (i + 1) * P], rhs=zr[:],
                        start=True, stop=True,
                    )
                    yb = work.tile([P, NODE_CHUNK], bf16, tag="yb")
                    nc.vector.tensor_scalar_add(yb[:], yp[:], b2_sb[:, i : i + 1])
                    outp = psB.tile([P, NODE_CHUNK], f32, tag="outp", bufs=1)
                    nc.tensor.matmul(
                        outp[:], lhsT=wo_sb[:, i * P : (i + 1) * P], rhs=yb[:],
                        start=True, stop=True,
                    )
                    nc.vector.tensor_tensor(
                        out=out_acc[:, csl], in0=out_acc[:, csl], in1=outp[:],
                        op=mybir.AluOpType.add,
                    )

            out_acc = cpool.tile([P, LOCAL], f32)
            nc.vector.memset(out_acc[:], 0.0)
            hT0 = cpool.tile([P, LOCAL], bf16)
            hT1 = cpool.tile([P, LOCAL], bf16)
            hT2 = cpool.tile([P, LOCAL], bf16)
            hT3 = cpool.tile([P, LOCAL], bf16)

            XB = 8  # node-tiles per xt load
            for nt in range(NWIN):
                sl = slice(nt * P, (nt + 1) * P)
                if nt % XB == 0:
                    bsl = slice(nt * P, (nt + XB) * P)
                    xt0 = work.tile([P, XB * P], bf16, tag="xt0")
                    nc.sync.dma_start(xt0[:], xt[0, :, bsl])
                    xt1 = work.tile([P, XB * P], bf16, tag="xt1")
                    nc.sync.dma_start(xt1[:], xt[1, :, bsl])
                j = (nt % XB) * P
                ps0 = psA.tile([P, NB * OUT_F], f32, tag="psw")
                nc.tensor.matmul(
                    ps0[:], lhsT=xt0[:, j : j + P], rhs=wb_sb[:, : NB * OUT_F],
                    start=True, stop=False,
                )
                nc.tensor.matmul(
                    ps0[:], lhsT=xt1[:, j : j + P], rhs=wb_sb[:, NB * OUT_F :],
                    start=False, stop=False,
                )
                nc.tensor.matmul(
                    ps0[:], lhsT=ones_sb[0:1, :], rhs=bb_sb[0:1, :],
                    start=False, stop=True,
                )
                # h0 cols -> transposed feature-major store (no inv_deg here)
                hsb = work.tile([P, OUT_F], f32r, tag="hsb")
                nc.vector.tensor_copy(hsb[:], ps0[:, :OUT_F])
                pst0 = psB.tile([P, P], f32r, tag="pst")
                nc.tensor.transpose(out=pst0[:], in_=hsb[:], identity=ident[:])
                nc.vector.tensor_copy(hT0[:, nt * P : (nt + 1) * P], pst0[:])
                # branch 1-3 cols -> f8 AG input chunk
                hsbb = work.tile([P, C1], f8, tag="hsbb")
                nc.vector.tensor_copy(hsbb[:], ps0[:, OUT_F:])
                nc.sync.dma_start(
                    hb1q[nt // WPC][(nt % WPC) * P : (nt % WPC + 1) * P, :C1], hsbb[:]
                )
                if nt % WPC == WPC - 1:
                    ag_chunk(hb1q, hg1, nt // WPC)

            mlp_branch(0, hT0)

            NPAIR = T // 2
            R8 = NCORES * QROW  # rows per AllGather chunk in hg

            def prop_pass(src_hg, C, CG, gdt, hTb, nxtq, nxt_hg, ndt):
                GMAX = 4  # 512-idx calls: 2 fit per 1024-desc ring -> drain never starves
                NT_TOT = NWIN * T
                NCALL = NT_TOT // GMAX
                LOOKAHEAD = 4
                # Gather calls are emitted in flat (chunk-staggered) order but
                # interleaved with the window loop so AllGather triggers for the
                # next pass sit in the gpsimd stream right after their flushes.
                chunks = [None] * NCALL
                emitted = 0

                def ensure_calls(upto):
                    nonlocal emitted
                    while emitted <= min(upto, NCALL - 1):
                        ci = emitted
                        t0 = ci * GMAX
                        g = gpool.tile([P, GMAX, CG], gdt, tag="gath")
                        nc.gpsimd.dma_gather(
                            g[:],
                            src_hg[0 : (call_kmax[ci] + 1) * R8, :],
                            gidx_sb[:, t0 * 8 : (t0 + GMAX) * 8],
                            GMAX * P,
                            GMAX * P,
                            CG,
                            queue_num=ci % 4,
                        )
                        chunks[ci] = g
                        emitted += 1

                maxcall = [
                    max(
                        (2 * fpos_flat[w * NPAIR + p] + 1) // GMAX
                        for p in range(NPAIR)
                    )
                    for w in range(NWIN)
                ]

                for w in range(NWIN):
                    ensure_calls(maxcall[w] + LOOKAHEAD)
                    oh = work.tile([P, T, P], gdt, tag="oh", bufs=6)
                    nc.vector.tensor_tensor(
                        out=oh[:],
                        in0=destl_sb[:, w * T : (w + 1) * T, None].to_broadcast(
                            [P, T, P]
                        ),
                        in1=iotab_sb[:, None, :].to_broadcast([P, T, P]),
                        op=mybir.AluOpType.is_equal,
                    )
                    psw = psA.tile([P, C], f32, tag="psw")
                    for p in range(NPAIR):
                        f = fpos_flat[w * NPAIR + p]
                        ci, sl = divmod(2 * f, GMAX)
                        nc.tensor.matmul(
                            psw[:],
                            lhsT=oh[:, 2 * p : 2 * p + 2, :],
                            rhs=chunks[ci][:, sl : sl + 2, :C],
                            start=(p == 0),
                            stop=(p == NPAIR - 1),
                            perf_mode=mybir.MatmulPerfMode.DoubleRow,
                        )
                    flush_mlp_cols(psw, 0, w, hTb)
                    if nxtq is not None:
                        scb = work.tile([P, C - P], ndt, tag="scb")
                        nc.vector.tensor_scalar_mul(
                            scb[:], psw[:, P:], invd_sb[:, w : w + 1]
                        )
                        nc.sync.dma_start(
                            nxtq[w // WPC][
                                (w % WPC) * P : (w % WPC + 1) * P, : C - P
                            ],
                            scb[:],
                        )
                        if w % WPC == WPC - 1:
                            ag_chunk(nxtq, nxt_hg, w // WPC)
                ensure_calls(NCALL - 1)

            prop_pass(hg1, C1, C1G, f8, hT1, hb2q, hg2, f8)
            mlp_branch(1, hT1)
            prop_pass(hg2, C2, C2, f8, hT2, hb3q, hg3, f8)
            mlp_branch(2, hT2)
            prop_pass(hg3, C3, C3G, f8, hT3, None, None, None)
            mlp_branch(3, hT3)

            for ch in range(LOCAL // NODE_CHUNK):
                csl = slice(ch * NODE_CHUNK, (ch + 1) * NODE_CHUNK)
                fin = work.tile([P, NODE_CHUNK], f32, tag="fin")
                nc.vector.tensor_scalar_add(fin[:], out_acc[:, csl], bo_sb[:, 0:1])
                nc.sync.dma_start(outT[:, csl], fin[:])

    nc.compile()
    return nc


def _preprocess(inputs):
    x = np.asarray(inputs["x"], dtype=np.float32)
    ei = np.asarray(inputs["edge_index"])
    row = ei[0].astype(np.int64)
    col = ei[1].astype(np.int64)

    deg = np.bincount(col, minlength=N).astype(np.float32)
    inv_deg = (1.0 / np.maximum(deg, 1.0)).astype(np.float32)

    order = np.argsort(col, kind="stable")
    rs = row[order]
    cs = col[order]
    wb_bounds = np.searchsorted(cs, np.arange(0, N + P, P))
    counts = np.diff(wb_bounds)
    T = int(np.ceil(counts.max() / P))
    T += T % 2  # DoubleRow matmuls consume tiles in pairs

    iota_row = np.tile(np.arange(P, dtype=np.float32)[None, :], (P, 1))
    iota_col = np.arange(P, dtype=np.float32)[:, None]
    import ml_dtypes
    iota_row_bf = iota_row.astype(ml_dtypes.bfloat16)

    Wb = np.asarray(inputs["Wb"], np.float32)
    bb = np.asarray(inputs["bb"], np.float32)
    W1 = np.asarray(inputs["W1"], np.float32)
    b1 = np.asarray(inputs["b1"], np.float32)
    W2 = np.asarray(inputs["W2"], np.float32)
    b2 = np.asarray(inputs["b2"], np.float32)
    Wo = np.asarray(inputs["Wo"], np.float32)
    bo = np.asarray(inputs["bo"], np.float32)
    bg = np.asarray(inputs["branch_gates"], np.float32)
    temp = np.asarray(inputs["temperature"], np.float32)

    g = bg / temp
    g = np.exp(g - g.max())
    gates = (g / g.sum()).astype(np.float32)

    bf = ml_dtypes.bfloat16
    wb_cat = np.concatenate([Wb[i] for i in range(NB)], axis=1)  # [256, 512]
    shared = {
        "wb": np.ascontiguousarray(wb_cat.reshape(2, P, NB * OUT_F)).astype(bf),
        "bbr": np.concatenate([bb[i] for i in range(NB)])[None, :].astype(bf),
        "w1s": np.concatenate([W1[i] for i in range(NB)], axis=1).astype(bf),
        "b1s": np.stack([b1[i] for i in range(NB)], axis=1).copy(),
        "w2s": np.concatenate([W2[i] for i in range(NB)], axis=1).astype(bf),
        "b2s": np.stack([b2[i] for i in range(NB)], axis=1).copy(),
        "wos": np.concatenate(
            [gates[i] * Wo[i * P : (i + 1) * P, :] for i in range(NB)], axis=1
        ).astype(bf),
        "boc": bo[:, None].copy(),
        "iota_t": iota_row,
        "iota_b": iota_row_bf,
        "iota_c": iota_col,
    }

    # Per-window edges sorted by AllGather chunk of their source row so early
    # tiles only depend on early AG chunks; kmax per DoubleRow pair records
    # the highest chunk a pair's real edges touch (pads -> chunk 0).
    NPAIR = T // 2
    kmaxp = np.zeros((NWIN, NPAIR), np.int64)
    cores_raw = []
    for c in range(NCORES):
        gidx_rows = np.zeros((NWIN, T * P), np.int16)
        dl = np.full((NWIN, T * P), -1.0, np.float32)
        for w in range(NWIN):
            gw = c * NWIN + w
            e0, e1 = wb_bounds[gw], wb_bounds[gw + 1]
            cnt = e1 - e0
            rr = rs[e0:e1]
            cw = cs[e0:e1] - gw * P
            ck = (rr % LOCAL) // 1024
            o2 = np.argsort(ck, kind="stable")
            rr, cw, ck = rr[o2], cw[o2], ck[o2]
            # row layout in hg after chunked AllGather: q-major, then rank
            rmap = ck * (NCORES * 1024) + (rr // LOCAL) * 1024 + rr % 1024
            gidx_rows[w, :cnt] = rmap.astype(np.int16)
            dl[w, :cnt] = cw.astype(np.float32)
            for p in range(NPAIR):
                if cnt > 256 * p:
                    last = min(256 * p + 255, cnt - 1)
                    kmaxp[w, p] = max(kmaxp[w, p], int(ck[last]))
        invd_sb = np.ascontiguousarray(
            inv_deg[c * LOCAL : (c + 1) * LOCAL].reshape(NWIN, P).T
        )
        xt_c = np.ascontiguousarray(
            x[c * LOCAL : (c + 1) * LOCAL].T.reshape(2, P, LOCAL)
        ).astype(ml_dtypes.bfloat16)
        cores_raw.append((gidx_rows, dl, invd_sb, xt_c))

    # Flat gather emission order: first STAG windows' pairs sorted by kmax so
    # the stream starts as soon as AG chunk 0 lands; rest window-major.
    STAG = 4
    stag = sorted(
        (int(kmaxp[w, p]), w, p) for w in range(STAG) for p in range(NPAIR)
    )
    order = [(w, p) for _, w, p in stag]
    order += [(w, p) for w in range(STAG, NWIN) for p in range(NPAIR)]
    fpos = {wp: i for i, wp in enumerate(order)}
    call_kmax = tuple(
        max(int(kmaxp[w, p]) for (w, p) in order[2 * ci : 2 * ci + 2])
        for ci in range(len(order) // 2)
    )
    meta = (T, tuple(fpos[(w, p)] for w in range(NWIN) for p in range(NPAIR)), call_kmax)

    NT_TOT = NWIN * T
    in_maps = []
    for c in range(NCORES):
        gidx_rows, dl, invd_sb, xt_c = cores_raw[c]
        gp = gidx_rows.reshape(NWIN, NPAIR, 2 * P)
        gidx_flat = np.stack([gp[w, p] for (w, p) in order]).reshape(NT_TOT, P)
        gi = gidx_flat.reshape(NT_TOT, 8, 16).transpose(2, 0, 1).reshape(16, NT_TOT * 8)
        gidx_sb = np.ascontiguousarray(np.tile(gi, (8, 1)))
        destl_sb = np.ascontiguousarray(
            dl.reshape(NWIN, T, P).transpose(2, 0, 1).reshape(P, NWIN * T)
        ).astype(ml_dtypes.bfloat16)
        m = dict(shared)
        m.update(gidx=gidx_sb, destl=destl_sb, invd=invd_sb, xt=xt_c)
        in_maps.append(m)
    return meta, in_maps


def kernel(**inputs) -> np.ndarray:
    global LAST_EXEC_NS
    from concourse.bass_utils import run_bass_kernel_spmd

    meta, in_maps = _preprocess(inputs)
    if meta not in _PROG_CACHE:
        _PROG_CACHE[meta] = _build_program(*meta)
    nc = _PROG_CACHE[meta]

    trace = bool(os.environ.get("KERNEL_TRACE"))
    if trace:
        _install_ntff_hook()
    res = run_bass_kernel_spmd(nc, in_maps, list(range(NCORES)), trace=trace)
    LAST_EXEC_NS = res.exec_time_ns

    out = np.empty((N, OUT_F), np.float32)
    for c in range(NCORES):
        out[c * LOCAL : (c + 1) * LOCAL, :] = res.results[c]["outT"].T
    return out

